# revision 6
# baseline (speedup 1.0000x reference)
"""Trainium2 Bass kernel for ConvMessageAggregator.

Computes, for each node n (messages: [N, 16, 688] fp32):
  f1[i] = relu(w10*x[i] + w11*x[i+2] + b1)      i in 0..13   (dilated 2-tap conv)
  f2[i] = relu(w20*f1[i] + w21*f1[i+2] + b2)    i in 0..11
  out   = relu(sum_k mlp_w[k] * f2[6+k] + mlp_b)             -> [N, 688]

Only f2 rows 6..11 are consumed, which depend on f1 rows 6..13, which depend
on x rows 6..15 -- so the kernel only reads the last 10 (contiguous) rows of
each node's 16-row block (10/16 of the input bytes).

Sharding: pure data parallel; node axis split across 8 NeuronCores, all
conv/MLP scalars baked into the instruction stream as immediates at trace
time (the program is rebuilt per call, so arbitrary weights are handled).

Per-core pipeline (2048 nodes = 16 tiles of 128 nodes on partitions):
  DMA  x[128, 10, 688]                                (HWDGE, one 3.5MB DMA)
  DVE  u1 = (x_other * r1) + x_pivot    [128, 8, 688] (scalar_tensor_tensor)
  ACT  f1 = Relu(p1*u1 + b1)            [128, 8, 688]
  DVE  u2 = (f1_other * r2) + f1_pivot  [128, 6, 688]
  ACT  f2 = Relu(p2*u2 + b2)            [128, 6, 688]
  DVE  5x binary-combine tree over the 6 rows (weight ratios all <= 1)
  ACT  out = Relu(w_anchor*t + mlp_b)   [128, 688]
  DMA  out tile -> DRAM
"""

import sys

for _p in ("/opt/trn_rl_repo",):
    if _p not in sys.path:
        sys.path.insert(0, _p)

import numpy as np

import concourse.bass as bass
import concourse.tile as tile
from concourse import mybir
from concourse.bass_utils import run_bass_kernel_spmd

N_FULL, L, MSG = 16384, 16, 688
N_CORES = 8
N_LOCAL = N_FULL // N_CORES  # 2048
P = 128                      # nodes per tile (partition dim)
NTILES = N_LOCAL // P        # 16
R0, NROWS = 6, 10            # input rows actually used: 6..15 (contiguous)

F32 = mybir.dt.float32
AF = mybir.ActivationFunctionType
OP = mybir.AluOpType


def _split_multi_waits(nc):
    """TPB instructions encode at most ONE semaphore wait; this walrus build's
    codegen rejects instructions with more. Hoist extra waits into standalone
    EventSemaphore ops on the same (in-order) sequencer -- semantically
    identical to the attached wait."""
    for func in nc.m.functions:
        for bb in func.blocks:
            insts = list(bb.instructions)
            if not any(
                i.sync_info is not None and len(i.sync_info.on_wait) > 1
                for i in insts
            ):
                continue
            new = []
            for inst in insts:
                si = inst.sync_info
                if si is not None and len(si.on_wait) > 1:
                    waits = list(si.on_wait)
                    for j, w in enumerate(waits[:-1]):
                        new.append(
                            mybir.InstEventSemaphore(
                                name=f"{inst.name}-hoistw{j}",
                                engine=inst.engine,
                                sync_info=mybir.SyncInfo(on_wait=[w], on_update=[]),
                            )
                        )
                    inst.sync_info = mybir.SyncInfo(
                        on_wait=[waits[-1]], on_update=list(si.on_update)
                    )
                new.append(inst)
            bb.instructions = new


def _conv_split(wa, wb):
    """Factor pre[i] = wa*in[i] + wb*in[i+2] as pivot*(in[pv] + r*in[ot]).

    Returns (pivot_weight, ratio, pivot_row_off, other_row_off) with |ratio|<=1.
    """
    if abs(wa) >= abs(wb):
        return wa, (wb / wa if wa != 0.0 else 0.0), 0, 2
    return wb, wa / wb, 2, 0


def build_program(w10, w11, b1, w20, w21, b2, mlp_w, mlp_b):
    nc = bass.Bass(trn_type="TRN2", name="conv_msg_agg")
    x = nc.dram_tensor("x", [N_LOCAL, L, MSG], F32, kind="ExternalInput")
    out = nc.dram_tensor("out", [N_LOCAL, MSG], F32, kind="ExternalOutput")

    p1, r1, pv1, ot1 = _conv_split(w10, w11)
    p2, r2, pv2, ot2 = _conv_split(w20, w21)

    with tile.TileContext(nc) as tc:
        with (
            tc.tile_pool(name="bias", bufs=1) as pool_b,
            tc.tile_pool(name="xin", bufs=2) as pool_x,
            tc.tile_pool(name="work", bufs=1) as pool_w,
            tc.tile_pool(name="tree", bufs=1) as pool_t,
            tc.tile_pool(name="outp", bufs=3) as pool_o,
        ):
            # activation() needs SBUF [P,1] bias vectors for non-Copy funcs
            b1c = pool_b.tile([P, 1], F32, tag="b1")
            nc.vector.memset(b1c[:], b1)
            b2c = pool_b.tile([P, 1], F32, tag="b2")
            nc.vector.memset(b2c[:], b2)
            mbc = pool_b.tile([P, 1], F32, tag="mb")
            nc.vector.memset(mbc[:], mlp_b)

            for it in range(NTILES):
                n0 = it * P
                xt = pool_x.tile([P, NROWS, MSG], F32, tag="x")
                nc.gpsimd.dma_start(out=xt[:], in_=x[n0 : n0 + P, R0 : R0 + NROWS, :])

                # conv1: 8 output rows from input rows 0..9 (of the loaded 10)
                f1 = pool_w.tile([P, 8, MSG], F32, tag="f1")
                if p1 == 0.0:
                    nc.vector.memset(f1[:], max(b1, 0.0))
                else:
                    u1 = pool_w.tile([P, 8, MSG], F32, tag="u1")
                    nc.vector.scalar_tensor_tensor(
                        out=u1[:],
                        in0=xt[:, ot1 : ot1 + 8, :],
                        scalar=r1,
                        in1=xt[:, pv1 : pv1 + 8, :],
                        op0=OP.mult,
                        op1=OP.add,
                    )
                    nc.scalar.activation(
                        out=f1[:], in_=u1[:], func=AF.Relu, bias=b1c[:], scale=p1
                    )

                # conv2: 6 output rows from f1 rows 0..7
                f2 = pool_w.tile([P, 6, MSG], F32, tag="f2")
                if p2 == 0.0:
                    nc.vector.memset(f2[:], max(b2, 0.0))
                else:
                    u2 = pool_w.tile([P, 6, MSG], F32, tag="u2")
                    nc.vector.scalar_tensor_tensor(
                        out=u2[:],
                        in0=f1[:, ot2 : ot2 + 6, :],
                        scalar=r2,
                        in1=f1[:, pv2 : pv2 + 6, :],
                        op0=OP.mult,
                        op1=OP.add,
                    )
                    nc.scalar.activation(
                        out=f2[:], in_=u2[:], func=AF.Relu, bias=b2c[:], scale=p2
                    )

                # weighted sum over the 6 f2 rows: pairwise combine, always
                # dividing by the larger weight so every immediate ratio <= 1.
                terms = [
                    (float(mlp_w[k]), f2[:, k, :]) for k in range(6) if mlp_w[k] != 0.0
                ]
                tcnt = 0
                while len(terms) > 1:
                    terms.sort(key=lambda t: -abs(t[0]))
                    nxt = []
                    for i in range(0, len(terms) - 1, 2):
                        wa, aa = terms[i]
                        wb, ab = terms[i + 1]
                        tt = pool_t.tile([P, MSG], F32, tag=f"t{tcnt}")
                        tcnt += 1
                        nc.vector.scalar_tensor_tensor(
                            out=tt[:],
                            in0=ab,
                            scalar=wb / wa,
                            in1=aa,
                            op0=OP.mult,
                            op1=OP.add,
                        )
                        nxt.append((wa, tt[:]))
                    if len(terms) % 2:
                        nxt.append(terms[-1])
                    terms = nxt

                ot = pool_o.tile([P, MSG], F32, tag="o")
                if terms:
                    wa, aa = terms[0]
                    nc.scalar.activation(
                        out=ot[:], in_=aa, func=AF.Relu, bias=mbc[:], scale=wa
                    )
                else:
                    nc.vector.memset(ot[:], max(mlp_b, 0.0))
                nc.gpsimd.dma_start(out=out[n0 : n0 + P, :], in_=ot[:])
    _split_multi_waits(nc)
    return nc


def run(inputs, trace=False, **spmd_kwargs):
    """Build + run on 8 cores. Returns (full_output, BassKernelResults)."""
    msgs = np.asarray(inputs["messages"], dtype=np.float32)
    assert msgs.shape == (N_FULL, L, MSG), msgs.shape
    if not msgs.flags["C_CONTIGUOUS"]:
        msgs = np.ascontiguousarray(msgs)

    c1w = np.asarray(inputs["conv1_w"], dtype=np.float64)
    c2w = np.asarray(inputs["conv2_w"], dtype=np.float64)
    mlw = np.asarray(inputs["mlp_w"], dtype=np.float64)
    nc = build_program(
        float(c1w[0]),
        float(c1w[1]),
        float(np.asarray(inputs["conv1_b"], dtype=np.float64)),
        float(c2w[0]),
        float(c2w[1]),
        float(np.asarray(inputs["conv2_b"], dtype=np.float64)),
        [float(v) for v in mlw],
        float(np.asarray(inputs["mlp_b"], dtype=np.float64)),
    )

    in_maps = [
        {"x": msgs[i * N_LOCAL : (i + 1) * N_LOCAL]} for i in range(N_CORES)
    ]
    res = run_bass_kernel_spmd(
        nc, in_maps, core_ids=list(range(N_CORES)), trace=trace, **spmd_kwargs
    )
    full = np.concatenate([r["out"] for r in res.results], axis=0)
    return full, res


def kernel(**inputs) -> np.ndarray:
    return run(inputs, trace=False)[0]
